# revision 7
# baseline (speedup 1.0000x reference)
"""Multi-head self-attention (no mask) on 8 TRN2 NeuronCores.

Problem: B=2, T=2048, C=1024, H=16 heads, D=64.
    q/k/v = x @ W{q,k,v}.T + b;  att = softmax(q k^T / sqrt(D));
    y = att v;  out = y @ Wp.T + bp.

Sharding: core (b, g) with b in {0,1} batches x g in {0..3} head-groups of 4
heads.  Each core computes q/k/v for its 4 heads over the full sequence of its
batch, attention for those heads, and the partial output projection through its
256 columns of Wp.  The host sums the 4 partial projections per batch and adds
bp (a pure post-add).  No device collectives needed.

On-core dataflow (everything f32r = TF32-class rounding on the PE; PSUM
accumulation is fp32):
  - x^T and W^T tiles produced via PE-transpose (fp32 DMA-transpose unsupported).
  - q^T/k^T [256, T] channel-on-partition; v [T, 256] natural with a ones
    column per head (65-wide groups) so that the y'-matmul also produces the
    softmax denominators as PSUM row 64.
  - S^T tile = k_h^T.T @ q_h^T (K=64 matmul); P = exp(S/8) on ACT straight out
    of PSUM; y'_h accumulated over 16 key tiles with V' as stationary.
  - normalization: DVE reciprocal of row 64, GPSIMD partition-broadcast,
    DVE multiply; odd heads partition-shifted into the packed y^T tile via
    SBUF->SBUF DMA (DVE cannot shift partitions).
  - out_partial = y^T.T @ Wp^T slice, written natural-layout.
"""

import sys
from contextlib import ExitStack

import numpy as np

if "/opt/trn_rl_repo" not in sys.path:
    sys.path.insert(0, "/opt/trn_rl_repo")

import concourse.bass as bass
import concourse.mybir as mybir
import concourse.tile as tile
from concourse import bacc
from concourse.bass_utils import run_bass_kernel_spmd
from concourse.masks import make_identity

F32 = mybir.dt.float32
F32R = mybir.dt.float32r
Act = mybir.ActivationFunctionType

P = 128
B, C, HEADS, D = 2, 1024, 16, 64
GROUPS = 4            # head groups (tensor-parallel dimension)
HLOC = HEADS // GROUPS  # 4 heads per core
G = HLOC * D          # 256 channels per core
KT = C // P           # 8 contraction tiles
VW = D + 1            # v group width incl. ones column


def build(T=2048, mm_dt=F32R, qk_dt=F32R, attn_dt=mybir.dt.bfloat16):
    """Build the per-core Bass program (identical on all 8 cores)."""
    TQ = 512            # query-chunk (matmul free dim)
    NTQ = T // TQ
    NS = T // P         # key tiles
    NXC = T // 256      # x-transpose chunks

    cast_needed = mm_dt != F32

    nc = bacc.Bacc("TRN2", target_bir_lowering=False, debug=False)
    x = nc.dram_tensor("x", [T, C], F32, kind="ExternalInput")
    wq = nc.dram_tensor("wq", [G, C], F32, kind="ExternalInput")
    wk = nc.dram_tensor("wk", [G, C], F32, kind="ExternalInput")
    wv = nc.dram_tensor("wv", [G, C], F32, kind="ExternalInput")
    wp = nc.dram_tensor("wp", [C, G], F32, kind="ExternalInput")
    bq = nc.dram_tensor("bq", [G], F32, kind="ExternalInput")
    bk = nc.dram_tensor("bk", [G], F32, kind="ExternalInput")
    bv = nc.dram_tensor("bv", [G], F32, kind="ExternalInput")
    out = nc.dram_tensor("out", [T, C], F32, kind="ExternalOutput")

    with tile.TileContext(nc) as tc, ExitStack() as ctx:
        persist = ctx.enter_context(tc.tile_pool(name="persist", bufs=1))

        ident = persist.tile([P, P], F32, tag="ident")
        make_identity(nc, ident[:])

        ones_row32 = persist.tile([1, P], F32, tag="ones_row32")
        nc.gpsimd.memset(ones_row32[:], 1.0)
        ones_row = persist.tile([1, P], mm_dt, tag="ones_row")
        nc.vector.tensor_copy(ones_row[:], ones_row32[:])

        ones4 = persist.tile([P, HLOC, 1], attn_dt, tag="ones4")
        nc.gpsimd.memset(ones4[:], 1.0)

        bq_pp = persist.tile([P, 2], F32, tag="bq_pp")
        bk_pp = persist.tile([P, 2], F32, tag="bk_pp")
        nc.sync.dma_start(bq_pp[:], bq[:].rearrange("(m p) -> p m", p=P))
        nc.sync.dma_start(bk_pp[:], bk[:].rearrange("(m p) -> p m", p=P))
        bv32 = persist.tile([1, G], F32, tag="bv32")
        nc.sync.dma_start(bv32[:], bv[None, :])
        bv_row = persist.tile([1, G], mm_dt, tag="bv_row")
        nc.vector.tensor_copy(bv_row[:], bv32[:])

        qT = persist.tile([P, 2, T], qk_dt, tag="qT")
        kT = persist.tile([P, 2, T], qk_dt, tag="kT")
        v_sb = persist.tile([P, NS, HLOC * VW], attn_dt, tag="v_sb")
        yT = persist.tile([P, 2, T], mm_dt, tag="yT")
        wpT = persist.tile([P, 2, C], mm_dt, tag="wpT")

        # ---------------- phase 1: transposes + QKV projections ----------------
        with (
            tc.tile_pool(name="xtp", bufs=1) as xtp,
            tc.tile_pool(name="wtp", bufs=1) as wtp,
            tc.tile_pool(name="stage", bufs=2) as stage,
            tc.tile_pool(name="ps1", bufs=2, space="PSUM") as ps1,
        ):
            xT = xtp.tile([P, KT, T], mm_dt, tag="xT")
            wqT = wtp.tile([P, KT, G], mm_dt, tag="wqT")
            wkT = wtp.tile([P, KT, G], mm_dt, tag="wkT")
            wvT = wtp.tile([P, KT, G], mm_dt, tag="wvT")

            # -- weight transposes: w [G, C] natural -> wT [C-tiles, G]
            for w_dram, wT in ((wq, wqT), (wk, wkT), (wv, wvT)):
                w_nat = stage.tile([P, 2, C], F32, tag="stg")
                nc.sync.dma_start(
                    w_nat[:], w_dram[:, :].rearrange("(a p) c -> p a c", p=P)
                )
                for ck in range(KT):
                    pt = ps1.tile([P, 2 * P], F32, tag="tr")
                    for j in range(2):
                        nc.tensor.transpose(
                            pt[:, j * P : (j + 1) * P],
                            w_nat[:, j, ck * P : (ck + 1) * P],
                            ident[:],
                        )
                    nc.vector.tensor_copy(wT[:, ck, :], pt[:])

            # -- wp transpose: wp [C, G] natural -> wpT [G-tiles, C]
            wp_nat = stage.tile([P, KT, G], F32, tag="stg")
            nc.sync.dma_start(
                wp_nat[:], wp[:, :].rearrange("(a p) g -> p a g", p=P)
            )
            for j in range(2):
                for ci in range(0, KT, 4):
                    pt4 = ps1.tile([P, 4 * P], F32, tag="tr")
                    for a in range(4):
                        nc.tensor.transpose(
                            pt4[:, a * P : (a + 1) * P],
                            wp_nat[:, ci + a, j * P : (j + 1) * P],
                            ident[:],
                        )
                    nc.vector.tensor_copy(
                        wpT[:, j, ci * P : (ci + 4) * P], pt4[:]
                    )

            # -- x transpose: x [T, C] -> xT [C-tiles, T], 256-row chunks
            for tch in range(NXC):
                x_nat = stage.tile([P, 2, C], F32, tag="stg")
                nc.sync.dma_start(
                    x_nat[:],
                    x[:, :].rearrange("(n a p) c -> n p a c", a=2, p=P)[tch],
                )
                for ck in range(KT):
                    pt = ps1.tile([P, 2 * P], F32, tag="tr")
                    for j in range(2):
                        nc.tensor.transpose(
                            pt[:, j * P : (j + 1) * P],
                            x_nat[:, j, ck * P : (ck + 1) * P],
                            ident[:],
                        )
                    nc.vector.tensor_copy(
                        xT[:, ck, 256 * tch : 256 * (tch + 1)], pt[:]
                    )

            # -- q^T / k^T projections: [G, T] channel-on-partition
            for wT, bias_pp, dstT in ((wqT, bq_pp, qT), (wkT, bk_pp, kT)):
                for m in range(2):
                    for tq in range(NTQ):
                        pq = ps1.tile([P, TQ], F32, tag="pq")
                        for kk in range(KT):
                            nc.tensor.matmul(
                                pq[:],
                                wT[:, kk, m * P : (m + 1) * P],
                                xT[:, kk, tq * TQ : (tq + 1) * TQ],
                                start=(kk == 0),
                                stop=(kk == KT - 1),
                            )
                        nc.scalar.activation(
                            dstT[:, m, tq * TQ : (tq + 1) * TQ],
                            pq[:],
                            Act.Identity,
                            bias=bias_pp[:, m : m + 1],
                            scale=1.0,
                        )

            # -- v projection, natural layout, ones column per head
            for s in range(NS):
                pv = ps1.tile([P, G], F32, tag="pv")
                for kk in range(KT):
                    nc.tensor.matmul(
                        pv[:],
                        xT[:, kk, s * P : (s + 1) * P],
                        wvT[:, kk, :],
                        start=(kk == 0),
                        stop=False,
                    )
                nc.tensor.matmul(
                    pv[:], ones_row[0:1, :], bv_row[0:1, :], start=False, stop=True
                )
                vs = v_sb[:, s, :].rearrange("p (h e) -> p h e", e=VW)
                nc.vector.tensor_copy(
                    vs[:, :, 0:D],
                    pv[:].rearrange("p (h d) -> p h d", d=D),
                )
                nc.vector.tensor_copy(vs[:, :, D : D + 1], ones4[:])

        # ---------------- phase 2: attention ----------------
        with (
            tc.tile_pool(name="ppool", bufs=4) as ppool,
            tc.tile_pool(name="npool", bufs=2) as npool,
            tc.tile_pool(name="sps", bufs=2, space="PSUM") as sps,
            tc.tile_pool(name="yps", bufs=1, space="PSUM") as yps,
            tc.tile_pool(name="opool", bufs=2) as opool,
        ):
            for pi in range(2):
                for tq in range(NTQ):
                    tqs = slice(tq * TQ, (tq + 1) * TQ)
                    py0 = yps.tile([VW, TQ], F32, tag="py0")
                    py1 = yps.tile([VW, TQ], F32, tag="py1")
                    py = [py0, py1]
                    for s in range(NS):
                        sp = sps.tile([P, 2 * TQ], F32, tag="sp")
                        for hh in range(2):
                            bp_ = 64 * hh
                            nc.tensor.matmul(
                                sp[:, hh * TQ : (hh + 1) * TQ],
                                kT[bp_ : bp_ + 64, pi, s * P : (s + 1) * P],
                                qT[bp_ : bp_ + 64, pi, tqs],
                                start=True,
                                stop=True,
                            )
                        pt = ppool.tile([P, 2 * TQ], attn_dt, tag="pt")
                        nc.scalar.activation(
                            pt[:], sp[:], Act.Exp, scale=1.0 / np.sqrt(D)
                        )
                        for hh in range(2):
                            h = 2 * pi + hh
                            nc.tensor.matmul(
                                py[hh][:],
                                v_sb[:, s, h * VW : (h + 1) * VW],
                                pt[:, hh * TQ : (hh + 1) * TQ],
                                start=(s == 0),
                                stop=(s == NS - 1),
                            )
                    # normalize: y_h / sums_h (sums in PSUM row 64)
                    for hh in range(2):
                        recip = npool.tile([VW, TQ], F32, tag=f"recip{hh}")
                        nc.vector.reciprocal(recip[D : D + 1, :], py[hh][D : D + 1, :])
                        # partition_broadcast reads the physical partition of
                        # its input AP's *tile base*, not the AP offset — move
                        # the reciprocal row to partition 0 first (DMA shifts
                        # partitions; DVE cannot).
                        recip0 = npool.tile([1, TQ], F32, tag=f"recip0{hh}")
                        nc.sync.dma_start(recip0[:], recip[D : D + 1, :])
                        bcast = npool.tile([D, TQ], F32, tag=f"bcast{hh}")
                        nc.gpsimd.partition_broadcast(
                            bcast[:, :], recip0[0:1, :], channels=D
                        )
                        if hh == 0:
                            nc.vector.tensor_mul(
                                yT[0:D, pi, tqs], py[hh][0:D, :], bcast[:, :]
                            )
                        else:
                            y_tmp = npool.tile([D, TQ], mm_dt, tag="y_tmp")
                            nc.vector.tensor_mul(
                                y_tmp[:], py[hh][0:D, :], bcast[:, :]
                            )
                            nc.sync.dma_start(yT[D : 2 * D, pi, tqs], y_tmp[:])

            # ---------------- phase 3: output projection (partial) ----------------
            for m in range(T // P):
                out_sb = opool.tile([P, C], F32, tag="osb")
                for n in range(2):
                    po = sps.tile([P, 512], F32, tag="po")
                    for j in range(2):
                        nc.tensor.matmul(
                            po[:],
                            yT[:, j, m * P : (m + 1) * P],
                            wpT[:, j, n * 512 : (n + 1) * 512],
                            start=(j == 0),
                            stop=(j == 1),
                        )
                    nc.vector.tensor_copy(out_sb[:, n * 512 : (n + 1) * 512], po[:])
                nc.sync.dma_start(out[m * P : (m + 1) * P, :], out_sb[:])

    nc.finalize()
    return nc


_NC_CACHE = {}


def _get_nc(T=2048):
    if T not in _NC_CACHE:
        _NC_CACHE[T] = build(T=T)
    return _NC_CACHE[T]


def _make_in_maps(x, Wq, bq, Wk, bk, Wv, bv, Wp):
    in_maps = []
    for b in range(B):
        xb = np.ascontiguousarray(x[b], dtype=np.float32)
        for g in range(GROUPS):
            sl = slice(g * G, (g + 1) * G)
            in_maps.append(
                {
                    "x": xb,
                    "wq": np.ascontiguousarray(Wq[sl, :], dtype=np.float32),
                    "wk": np.ascontiguousarray(Wk[sl, :], dtype=np.float32),
                    "wv": np.ascontiguousarray(Wv[sl, :], dtype=np.float32),
                    "wp": np.ascontiguousarray(Wp[:, sl], dtype=np.float32),
                    "bq": np.ascontiguousarray(bq[sl], dtype=np.float32),
                    "bk": np.ascontiguousarray(bk[sl], dtype=np.float32),
                    "bv": np.ascontiguousarray(bv[sl], dtype=np.float32),
                }
            )
    return in_maps


def run(inputs, trace=False):
    """Run on 8 cores; returns (out [B,T,C] fp32, BassKernelResults)."""
    x = np.asarray(inputs["x"], dtype=np.float32)
    T = x.shape[1]
    in_maps = _make_in_maps(
        x,
        np.asarray(inputs["Wq"]), np.asarray(inputs["bq"]),
        np.asarray(inputs["Wk"]), np.asarray(inputs["bk"]),
        np.asarray(inputs["Wv"]), np.asarray(inputs["bv"]),
        np.asarray(inputs["Wp"]),
    )
    nc = _get_nc(T)
    res = run_bass_kernel_spmd(
        nc, in_maps, core_ids=list(range(B * GROUPS)), trace=trace
    )
    bp = np.asarray(inputs["bp"], dtype=np.float32)
    parts = [res.results[i]["out"] for i in range(B * GROUPS)]
    out = np.stack(
        [sum(parts[b * GROUPS : (b + 1) * GROUPS]) for b in range(B)]
    ) + bp[None, None, :]
    return out.astype(np.float32), res


def kernel(**inputs):
    out, _ = run(inputs, trace=False)
    return out


# revision 10
# speedup vs baseline: 1.2254x; 1.2254x over previous
"""Multi-head self-attention (no mask) on 8 TRN2 NeuronCores.

Problem: B=2, T=2048, C=1024, H=16 heads, D=64.
    q/k/v = x @ W{q,k,v}.T + b;  att = softmax(q k^T / sqrt(D));
    y = att v;  out = y @ Wp.T + bp.

Sharding: core (b, g) with b in {0,1} batches x g in {0..3} head-groups of 4
heads.  Each core computes q/k/v for its 4 heads over the full sequence of its
batch, attention for those heads, and the partial output projection through its
256 columns of Wp.  The host sums the 4 partial projections per batch and adds
bp (a pure post-add).  No device collectives needed.

On-core dataflow (everything f32r = TF32-class rounding on the PE; PSUM
accumulation is fp32):
  - x^T and W^T tiles produced via PE-transpose (fp32 DMA-transpose unsupported).
  - q^T/k^T [256, T] channel-on-partition; v [T, 256] natural with a ones
    column per head (65-wide groups) so that the y'-matmul also produces the
    softmax denominators as PSUM row 64.
  - S^T tile = k_h^T.T @ q_h^T (K=64 matmul); P = exp(S/8) on ACT straight out
    of PSUM; y'_h accumulated over 16 key tiles with V' as stationary.
  - normalization: DVE reciprocal of row 64, GPSIMD partition-broadcast,
    DVE multiply; odd heads partition-shifted into the packed y^T tile via
    SBUF->SBUF DMA (DVE cannot shift partitions).
  - out_partial = y^T.T @ Wp^T slice, written natural-layout.
"""

import sys
from contextlib import ExitStack

import numpy as np

if "/opt/trn_rl_repo" not in sys.path:
    sys.path.insert(0, "/opt/trn_rl_repo")

import concourse.bass as bass
import concourse.mybir as mybir
import concourse.tile as tile
from concourse import bacc
from concourse.bass_utils import run_bass_kernel_spmd
from concourse.masks import make_identity

F32 = mybir.dt.float32
F32R = mybir.dt.float32r
Act = mybir.ActivationFunctionType

P = 128
B, C, HEADS, D = 2, 1024, 16, 64
GROUPS = 4            # head groups (tensor-parallel dimension)
HLOC = HEADS // GROUPS  # 4 heads per core
G = HLOC * D          # 256 channels per core
KT = C // P           # 8 contraction tiles
VW = D + 1            # v group width incl. ones column


def build(T=2048, mm_dt=F32R, qk_dt=F32R, attn_dt=mybir.dt.bfloat16):
    """Build the per-core Bass program (identical on all 8 cores)."""
    TQ = 512            # query-chunk (matmul free dim)
    NTQ = T // TQ
    NS = T // P         # key tiles
    NXC = T // 256      # x-transpose chunks

    cast_needed = mm_dt != F32

    nc = bacc.Bacc("TRN2", target_bir_lowering=False, debug=False)
    x = nc.dram_tensor("x", [T, C], F32, kind="ExternalInput")
    wq = nc.dram_tensor("wq", [G, C], F32, kind="ExternalInput")
    wk = nc.dram_tensor("wk", [G, C], F32, kind="ExternalInput")
    wv = nc.dram_tensor("wv", [G, C], F32, kind="ExternalInput")
    wp = nc.dram_tensor("wp", [C, G], F32, kind="ExternalInput")
    bq = nc.dram_tensor("bq", [G], F32, kind="ExternalInput")
    bk = nc.dram_tensor("bk", [G], F32, kind="ExternalInput")
    bv = nc.dram_tensor("bv", [G], F32, kind="ExternalInput")
    out = nc.dram_tensor("out", [T, C], F32, kind="ExternalOutput")

    with tile.TileContext(nc) as tc, ExitStack() as ctx:
        persist = ctx.enter_context(tc.tile_pool(name="persist", bufs=1))

        ident = persist.tile([P, P], F32, tag="ident")
        make_identity(nc, ident[:])

        ones_row32 = persist.tile([1, P], F32, tag="ones_row32")
        nc.gpsimd.memset(ones_row32[:], 1.0)
        ones_row = persist.tile([1, P], mm_dt, tag="ones_row")
        nc.vector.tensor_copy(ones_row[:], ones_row32[:])

        ones4 = persist.tile([P, HLOC, 1], attn_dt, tag="ones4")
        nc.gpsimd.memset(ones4[:], 1.0)

        bq_pp = persist.tile([P, 2], F32, tag="bq_pp")
        bk_pp = persist.tile([P, 2], F32, tag="bk_pp")
        nc.sync.dma_start(bq_pp[:], bq[:].rearrange("(m p) -> p m", p=P))
        nc.sync.dma_start(bk_pp[:], bk[:].rearrange("(m p) -> p m", p=P))
        bv32 = persist.tile([1, G], F32, tag="bv32")
        nc.sync.dma_start(bv32[:], bv[None, :])
        bv_row = persist.tile([1, G], mm_dt, tag="bv_row")
        nc.vector.tensor_copy(bv_row[:], bv32[:])

        qT = persist.tile([P, 2, T], qk_dt, tag="qT")
        kT = persist.tile([P, 2, T], qk_dt, tag="kT")
        v_sb = persist.tile([P, NS, HLOC * VW], attn_dt, tag="v_sb")
        yT = persist.tile([P, 2, T], mm_dt, tag="yT")
        wpT = persist.tile([P, 2, C], mm_dt, tag="wpT")

        # ---------------- phase 1: transposes + QKV projections ----------------
        with (
            tc.tile_pool(name="xtp", bufs=1) as xtp,
            tc.tile_pool(name="wtp", bufs=1) as wtp,
            tc.tile_pool(name="stage", bufs=2) as stage,
            tc.tile_pool(name="ps1", bufs=2, space="PSUM") as ps1,
        ):
            xT = xtp.tile([P, KT, T], mm_dt, tag="xT")
            wqT = wtp.tile([P, KT, G], mm_dt, tag="wqT")
            wkT = wtp.tile([P, KT, G], mm_dt, tag="wkT")
            wvT = wtp.tile([P, KT, G], mm_dt, tag="wvT")

            # -- weight transposes: w [G, C] natural -> wT [C-tiles, G]
            for w_dram, wT in ((wq, wqT), (wk, wkT), (wv, wvT)):
                w_nat = stage.tile([P, 2, C], F32, tag="stg")
                nc.sync.dma_start(
                    w_nat[:], w_dram[:, :].rearrange("(a p) c -> p a c", p=P)
                )
                for ck in range(KT):
                    pt = ps1.tile([P, 2 * P], F32, tag="tr")
                    for j in range(2):
                        nc.tensor.transpose(
                            pt[:, j * P : (j + 1) * P],
                            w_nat[:, j, ck * P : (ck + 1) * P],
                            ident[:],
                        )
                    nc.vector.tensor_copy(wT[:, ck, :], pt[:])

            # -- wp transpose: wp [C, G] natural -> wpT [G-tiles, C]
            wp_nat = stage.tile([P, KT, G], F32, tag="stg")
            nc.sync.dma_start(
                wp_nat[:], wp[:, :].rearrange("(a p) g -> p a g", p=P)
            )
            for j in range(2):
                for ci in range(0, KT, 4):
                    pt4 = ps1.tile([P, 4 * P], F32, tag="tr")
                    for a in range(4):
                        nc.tensor.transpose(
                            pt4[:, a * P : (a + 1) * P],
                            wp_nat[:, ci + a, j * P : (j + 1) * P],
                            ident[:],
                        )
                    nc.vector.tensor_copy(
                        wpT[:, j, ci * P : (ci + 4) * P], pt4[:]
                    )

            # -- x transpose: x [T, C] -> xT [C-tiles, T], 256-row chunks
            for tch in range(NXC):
                x_nat = stage.tile([P, 2, C], F32, tag="stg")
                nc.sync.dma_start(
                    x_nat[:],
                    x[:, :].rearrange("(n a p) c -> n p a c", a=2, p=P)[tch],
                )
                for ck in range(KT):
                    pt = ps1.tile([P, 2 * P], F32, tag="tr")
                    for j in range(2):
                        nc.tensor.transpose(
                            pt[:, j * P : (j + 1) * P],
                            x_nat[:, j, ck * P : (ck + 1) * P],
                            ident[:],
                        )
                    nc.vector.tensor_copy(
                        xT[:, ck, 256 * tch : 256 * (tch + 1)], pt[:]
                    )

            # -- q^T / k^T projections: [G, T] channel-on-partition
            for wT, bias_pp, dstT in ((wqT, bq_pp, qT), (wkT, bk_pp, kT)):
                for m in range(2):
                    for tq in range(NTQ):
                        pq = ps1.tile([P, TQ], F32, tag="pq")
                        for kk in range(KT):
                            nc.tensor.matmul(
                                pq[:],
                                wT[:, kk, m * P : (m + 1) * P],
                                xT[:, kk, tq * TQ : (tq + 1) * TQ],
                                start=(kk == 0),
                                stop=(kk == KT - 1),
                            )
                        nc.scalar.activation(
                            dstT[:, m, tq * TQ : (tq + 1) * TQ],
                            pq[:],
                            Act.Identity,
                            bias=bias_pp[:, m : m + 1],
                            scale=1.0,
                        )

            # -- v projection, natural layout, ones column per head
            for s in range(NS):
                pv = ps1.tile([P, G], F32, tag="pv")
                for kk in range(KT):
                    nc.tensor.matmul(
                        pv[:],
                        xT[:, kk, s * P : (s + 1) * P],
                        wvT[:, kk, :],
                        start=(kk == 0),
                        stop=False,
                    )
                nc.tensor.matmul(
                    pv[:], ones_row[0:1, :], bv_row[0:1, :], start=False, stop=True
                )
                vs = v_sb[:, s, :].rearrange("p (h e) -> p h e", e=VW)
                nc.vector.tensor_copy(
                    vs[:, :, 0:D],
                    pv[:].rearrange("p (h d) -> p h d", d=D),
                )
                nc.vector.tensor_copy(vs[:, :, D : D + 1], ones4[:])

        # ---------------- phase 2: attention ----------------
        with (
            tc.tile_pool(name="ppool", bufs=4) as ppool,
            tc.tile_pool(name="npool", bufs=2) as npool,
            tc.tile_pool(name="sps", bufs=2, space="PSUM") as sps,
            tc.tile_pool(name="yps", bufs=2, space="PSUM") as yps,
        ):
            for pi in range(2):
                for tq in range(NTQ):
                    tqs = slice(tq * TQ, (tq + 1) * TQ)
                    py0 = yps.tile([VW, TQ], F32, tag="py0")
                    py1 = yps.tile([VW, TQ], F32, tag="py1")
                    py = [py0, py1]
                    for s in range(NS):
                        sp = sps.tile([P, 2 * TQ], F32, tag="sp")
                        for hh in range(2):
                            bp_ = 64 * hh
                            nc.tensor.matmul(
                                sp[:, hh * TQ : (hh + 1) * TQ],
                                kT[bp_ : bp_ + 64, pi, s * P : (s + 1) * P],
                                qT[bp_ : bp_ + 64, pi, tqs],
                                start=True,
                                stop=True,
                            )
                        pt = ppool.tile([P, 2 * TQ], attn_dt, tag="pt")
                        nc.scalar.activation(
                            pt[:], sp[:], Act.Exp, scale=1.0 / np.sqrt(D)
                        )
                        for hh in range(2):
                            h = 2 * pi + hh
                            nc.tensor.matmul(
                                py[hh][:],
                                v_sb[:, s, h * VW : (h + 1) * VW],
                                pt[:, hh * TQ : (hh + 1) * TQ],
                                start=(s == 0),
                                stop=(s == NS - 1),
                            )
                    # normalize: y_h / sums_h (sums in PSUM row 64)
                    for hh in range(2):
                        recip = npool.tile([VW, TQ], F32, tag=f"recip{hh}")
                        nc.vector.reciprocal(recip[D : D + 1, :], py[hh][D : D + 1, :])
                        # partition_broadcast reads the physical partition of
                        # its input AP's *tile base*, not the AP offset — move
                        # the reciprocal row to partition 0 first (DMA shifts
                        # partitions; DVE cannot).
                        recip0 = npool.tile([1, TQ], F32, tag=f"recip0{hh}")
                        nc.sync.dma_start(recip0[:], recip[D : D + 1, :])
                        bcast = npool.tile([D, TQ], F32, tag=f"bcast{hh}")
                        nc.gpsimd.partition_broadcast(
                            bcast[:, :], recip0[0:1, :], channels=D
                        )
                        if hh == 0:
                            nc.vector.tensor_mul(
                                yT[0:D, pi, tqs], py[hh][0:D, :], bcast[:, :]
                            )
                        else:
                            y_tmp = npool.tile([D, TQ], mm_dt, tag="y_tmp")
                            nc.vector.tensor_mul(
                                y_tmp[:], py[hh][0:D, :], bcast[:, :]
                            )
                            nc.sync.dma_start(yT[D : 2 * D, pi, tqs], y_tmp[:])

        # ---------------- phase 3: output projection (partial) ----------------
        with (
            tc.tile_pool(name="ops2", bufs=3, space="PSUM") as ops2,
            tc.tile_pool(name="opool", bufs=3) as opool,
        ):
            for m in range(T // P):
                out_sb = opool.tile([P, C], F32, tag="osb")
                for n in range(2):
                    po = ops2.tile([P, 512], F32, tag="po")
                    for j in range(2):
                        nc.tensor.matmul(
                            po[:],
                            yT[:, j, m * P : (m + 1) * P],
                            wpT[:, j, n * 512 : (n + 1) * 512],
                            start=(j == 0),
                            stop=(j == 1),
                        )
                    nc.vector.tensor_copy(out_sb[:, n * 512 : (n + 1) * 512], po[:])
                nc.sync.dma_start(out[m * P : (m + 1) * P, :], out_sb[:])

    nc.finalize()
    return nc


_NC_CACHE = {}


def _get_nc(T=2048):
    if T not in _NC_CACHE:
        _NC_CACHE[T] = build(T=T)
    return _NC_CACHE[T]


def _make_in_maps(x, Wq, bq, Wk, bk, Wv, bv, Wp):
    in_maps = []
    for b in range(B):
        xb = np.ascontiguousarray(x[b], dtype=np.float32)
        for g in range(GROUPS):
            sl = slice(g * G, (g + 1) * G)
            in_maps.append(
                {
                    "x": xb,
                    "wq": np.ascontiguousarray(Wq[sl, :], dtype=np.float32),
                    "wk": np.ascontiguousarray(Wk[sl, :], dtype=np.float32),
                    "wv": np.ascontiguousarray(Wv[sl, :], dtype=np.float32),
                    "wp": np.ascontiguousarray(Wp[:, sl], dtype=np.float32),
                    "bq": np.ascontiguousarray(bq[sl], dtype=np.float32),
                    "bk": np.ascontiguousarray(bk[sl], dtype=np.float32),
                    "bv": np.ascontiguousarray(bv[sl], dtype=np.float32),
                }
            )
    return in_maps


def run(inputs, trace=False):
    """Run on 8 cores; returns (out [B,T,C] fp32, BassKernelResults)."""
    x = np.asarray(inputs["x"], dtype=np.float32)
    T = x.shape[1]
    in_maps = _make_in_maps(
        x,
        np.asarray(inputs["Wq"]), np.asarray(inputs["bq"]),
        np.asarray(inputs["Wk"]), np.asarray(inputs["bk"]),
        np.asarray(inputs["Wv"]), np.asarray(inputs["bv"]),
        np.asarray(inputs["Wp"]),
    )
    nc = _get_nc(T)
    res = run_bass_kernel_spmd(
        nc, in_maps, core_ids=list(range(B * GROUPS)), trace=trace
    )
    bp = np.asarray(inputs["bp"], dtype=np.float32)
    parts = [res.results[i]["out"] for i in range(B * GROUPS)]
    out = np.stack(
        [sum(parts[b * GROUPS : (b + 1) * GROUPS]) for b in range(B)]
    ) + bp[None, None, :]
    return out.astype(np.float32), res


def kernel(**inputs):
    out, _ = run(inputs, trace=False)
    return out


# revision 11
# speedup vs baseline: 1.2429x; 1.0143x over previous
"""Multi-head self-attention (no mask) on 8 TRN2 NeuronCores.

Problem: B=2, T=2048, C=1024, H=16 heads, D=64.
    q/k/v = x @ W{q,k,v}.T + b;  att = softmax(q k^T / sqrt(D));
    y = att v;  out = y @ Wp.T + bp.

Sharding: core (b, g) with b in {0,1} batches x g in {0..3} head-groups of 4
heads.  Each core computes q/k/v for its 4 heads over the full sequence of its
batch, attention for those heads, and the partial output projection through its
256 columns of Wp.  The host sums the 4 partial projections per batch and adds
bp (a pure post-add).  No device collectives needed.

On-core dataflow (everything f32r = TF32-class rounding on the PE; PSUM
accumulation is fp32):
  - x^T and W^T tiles produced via PE-transpose (fp32 DMA-transpose unsupported).
  - q^T/k^T [256, T] channel-on-partition; v [T, 256] natural with a ones
    column per head (65-wide groups) so that the y'-matmul also produces the
    softmax denominators as PSUM row 64.
  - S^T tile = k_h^T.T @ q_h^T (K=64 matmul); P = exp(S/8) on ACT straight out
    of PSUM; y'_h accumulated over 16 key tiles with V' as stationary.
  - normalization: DVE reciprocal of row 64, GPSIMD partition-broadcast,
    DVE multiply; odd heads partition-shifted into the packed y^T tile via
    SBUF->SBUF DMA (DVE cannot shift partitions).
  - out_partial = y^T.T @ Wp^T slice, written natural-layout.
"""

import sys
from contextlib import ExitStack

import numpy as np

if "/opt/trn_rl_repo" not in sys.path:
    sys.path.insert(0, "/opt/trn_rl_repo")

import concourse.bass as bass
import concourse.mybir as mybir
import concourse.tile as tile
from concourse import bacc
from concourse.bass_utils import run_bass_kernel_spmd
from concourse.masks import make_identity

F32 = mybir.dt.float32
F32R = mybir.dt.float32r
Act = mybir.ActivationFunctionType

P = 128
B, C, HEADS, D = 2, 1024, 16, 64
GROUPS = 4            # head groups (tensor-parallel dimension)
HLOC = HEADS // GROUPS  # 4 heads per core
G = HLOC * D          # 256 channels per core
KT = C // P           # 8 contraction tiles
VW = D + 1            # v group width incl. ones column


def build(T=2048, mm_dt=F32R, qk_dt=F32R, attn_dt=mybir.dt.bfloat16):
    """Build the per-core Bass program (identical on all 8 cores)."""
    TQ = 512            # query-chunk (matmul free dim)
    NTQ = T // TQ
    NS = T // P         # key tiles
    NXC = T // 256      # x-transpose chunks

    cast_needed = mm_dt != F32

    nc = bacc.Bacc("TRN2", target_bir_lowering=False, debug=False)
    x = nc.dram_tensor("x", [T, C], F32, kind="ExternalInput")
    wq = nc.dram_tensor("wq", [G, C], F32, kind="ExternalInput")
    wk = nc.dram_tensor("wk", [G, C], F32, kind="ExternalInput")
    wv = nc.dram_tensor("wv", [G, C], F32, kind="ExternalInput")
    wp = nc.dram_tensor("wp", [C, G], F32, kind="ExternalInput")
    bq = nc.dram_tensor("bq", [G], F32, kind="ExternalInput")
    bk = nc.dram_tensor("bk", [G], F32, kind="ExternalInput")
    bv = nc.dram_tensor("bv", [G], F32, kind="ExternalInput")
    out = nc.dram_tensor("out", [T, C], F32, kind="ExternalOutput")

    with tile.TileContext(nc) as tc, ExitStack() as ctx:
        persist = ctx.enter_context(tc.tile_pool(name="persist", bufs=1))

        ident = persist.tile([P, P], F32, tag="ident")
        make_identity(nc, ident[:])

        ones_row32 = persist.tile([1, P], F32, tag="ones_row32")
        nc.gpsimd.memset(ones_row32[:], 1.0)
        ones_row = persist.tile([1, P], mm_dt, tag="ones_row")
        nc.vector.tensor_copy(ones_row[:], ones_row32[:])

        ones4 = persist.tile([P, HLOC, 1], attn_dt, tag="ones4")
        nc.gpsimd.memset(ones4[:], 1.0)

        bq_pp = persist.tile([P, 2], F32, tag="bq_pp")
        bk_pp = persist.tile([P, 2], F32, tag="bk_pp")
        nc.sync.dma_start(bq_pp[:], bq[:].rearrange("(m p) -> p m", p=P))
        nc.sync.dma_start(bk_pp[:], bk[:].rearrange("(m p) -> p m", p=P))
        bv32 = persist.tile([1, G], F32, tag="bv32")
        nc.sync.dma_start(bv32[:], bv[None, :])
        bv_row = persist.tile([1, G], mm_dt, tag="bv_row")
        nc.vector.tensor_copy(bv_row[:], bv32[:])

        qT = persist.tile([P, 2, T], qk_dt, tag="qT")
        kT = persist.tile([P, 2, T], qk_dt, tag="kT")
        v_sb = persist.tile([P, NS, HLOC * VW], attn_dt, tag="v_sb")
        yT = persist.tile([P, 2, T], mm_dt, tag="yT")
        wpT = persist.tile([P, 2, C], mm_dt, tag="wpT")

        # ---------------- phase 1: transposes + QKV projections ----------------
        with (
            tc.tile_pool(name="xtp", bufs=1) as xtp,
            tc.tile_pool(name="wtp", bufs=1) as wtp,
            tc.tile_pool(name="stage", bufs=2) as stage,
            tc.tile_pool(name="ps1", bufs=2, space="PSUM") as ps1,
        ):
            xT = xtp.tile([P, KT, T], mm_dt, tag="xT")
            wqT = wtp.tile([P, KT, G], mm_dt, tag="wqT")
            wkT = wtp.tile([P, KT, G], mm_dt, tag="wkT")
            wvT = wtp.tile([P, KT, G], mm_dt, tag="wvT")

            # -- weight transposes: w [G, C] natural -> wT [C-tiles, G]
            for w_dram, wT in ((wq, wqT), (wk, wkT), (wv, wvT)):
                w_nat = stage.tile([P, 2, C], F32, tag="stg")
                nc.sync.dma_start(
                    w_nat[:], w_dram[:, :].rearrange("(a p) c -> p a c", p=P)
                )
                for ck in range(KT):
                    pt = ps1.tile([P, 2 * P], F32, tag="tr")
                    for j in range(2):
                        nc.tensor.transpose(
                            pt[:, j * P : (j + 1) * P],
                            w_nat[:, j, ck * P : (ck + 1) * P],
                            ident[:],
                        )
                    nc.vector.tensor_copy(wT[:, ck, :], pt[:])

            # -- wp transpose: wp [C, G] natural -> wpT [G-tiles, C]
            wp_nat = stage.tile([P, KT, G], F32, tag="stg")
            nc.sync.dma_start(
                wp_nat[:], wp[:, :].rearrange("(a p) g -> p a g", p=P)
            )
            for j in range(2):
                for ci in range(0, KT, 4):
                    pt4 = ps1.tile([P, 4 * P], F32, tag="tr")
                    for a in range(4):
                        nc.tensor.transpose(
                            pt4[:, a * P : (a + 1) * P],
                            wp_nat[:, ci + a, j * P : (j + 1) * P],
                            ident[:],
                        )
                    nc.vector.tensor_copy(
                        wpT[:, j, ci * P : (ci + 4) * P], pt4[:]
                    )

            # -- x transpose: x [T, C] -> xT [C-tiles, T], 256-row chunks
            for tch in range(NXC):
                x_nat = stage.tile([P, 2, C], F32, tag="stg")
                nc.sync.dma_start(
                    x_nat[:],
                    x[:, :].rearrange("(n a p) c -> n p a c", a=2, p=P)[tch],
                )
                for ck in range(KT):
                    pt = ps1.tile([P, 2 * P], F32, tag="tr")
                    for j in range(2):
                        nc.tensor.transpose(
                            pt[:, j * P : (j + 1) * P],
                            x_nat[:, j, ck * P : (ck + 1) * P],
                            ident[:],
                        )
                    nc.vector.tensor_copy(
                        xT[:, ck, 256 * tch : 256 * (tch + 1)], pt[:]
                    )

            # -- v projection, natural layout, ones column per head
            for s in range(NS):
                pv = ps1.tile([P, G], F32, tag="pv")
                for kk in range(KT):
                    nc.tensor.matmul(
                        pv[:],
                        xT[:, kk, s * P : (s + 1) * P],
                        wvT[:, kk, :],
                        start=(kk == 0),
                        stop=False,
                    )
                nc.tensor.matmul(
                    pv[:], ones_row[0:1, :], bv_row[0:1, :], start=False, stop=True
                )
                vs = v_sb[:, s, :].rearrange("p (h e) -> p h e", e=VW)
                nc.vector.tensor_copy(
                    vs[:, :, 0:D],
                    pv[:].rearrange("p (h d) -> p h d", d=D),
                )
                nc.vector.tensor_copy(vs[:, :, D : D + 1], ones4[:])

            # -- q^T / k^T projections: [G, T] channel-on-partition
            # (emitted after v, grouped by head-pair m so attention on pair 0
            # can start while pair 1 still projects)
            for m in range(2):
                for wT, bias_pp, dstT in ((wqT, bq_pp, qT), (wkT, bk_pp, kT)):
                    for tq in range(NTQ):
                        pq = ps1.tile([P, TQ], F32, tag="pq")
                        for kk in range(KT):
                            nc.tensor.matmul(
                                pq[:],
                                wT[:, kk, m * P : (m + 1) * P],
                                xT[:, kk, tq * TQ : (tq + 1) * TQ],
                                start=(kk == 0),
                                stop=(kk == KT - 1),
                            )
                        nc.scalar.activation(
                            dstT[:, m, tq * TQ : (tq + 1) * TQ],
                            pq[:],
                            Act.Identity,
                            bias=bias_pp[:, m : m + 1],
                            scale=1.0,
                        )

        # ---------------- phase 2: attention ----------------
        with (
            tc.tile_pool(name="ppool", bufs=4) as ppool,
            tc.tile_pool(name="npool", bufs=2) as npool,
            tc.tile_pool(name="sps", bufs=2, space="PSUM") as sps,
            tc.tile_pool(name="yps", bufs=2, space="PSUM") as yps,
        ):
            for pi in range(2):
                for tq in range(NTQ):
                    tqs = slice(tq * TQ, (tq + 1) * TQ)
                    py0 = yps.tile([VW, TQ], F32, tag="py0")
                    py1 = yps.tile([VW, TQ], F32, tag="py1")
                    py = [py0, py1]
                    for s in range(NS):
                        sp = sps.tile([P, 2 * TQ], F32, tag="sp")
                        for hh in range(2):
                            bp_ = 64 * hh
                            nc.tensor.matmul(
                                sp[:, hh * TQ : (hh + 1) * TQ],
                                kT[bp_ : bp_ + 64, pi, s * P : (s + 1) * P],
                                qT[bp_ : bp_ + 64, pi, tqs],
                                start=True,
                                stop=True,
                            )
                        pt = ppool.tile([P, 2 * TQ], attn_dt, tag="pt")
                        nc.scalar.activation(
                            pt[:], sp[:], Act.Exp, scale=1.0 / np.sqrt(D)
                        )
                        for hh in range(2):
                            h = 2 * pi + hh
                            nc.tensor.matmul(
                                py[hh][:],
                                v_sb[:, s, h * VW : (h + 1) * VW],
                                pt[:, hh * TQ : (hh + 1) * TQ],
                                start=(s == 0),
                                stop=(s == NS - 1),
                            )
                    # normalize: y_h / sums_h (sums in PSUM row 64)
                    for hh in range(2):
                        # sums row lives at PSUM partition 64; the custom-DVE
                        # reciprocal and gpsimd broadcast both require
                        # partition-0 inputs (they ignore AP partition
                        # offsets on HW), so: DVE copy (aligned) -> DMA
                        # partition-shift -> approx reciprocal at base 0.
                        srow = npool.tile([VW, TQ], F32, tag=f"srow{hh}")
                        nc.vector.tensor_copy(srow[D : D + 1, :], py[hh][D : D + 1, :])
                        srow0 = npool.tile([1, TQ], F32, tag=f"srow0{hh}")
                        nc.sync.dma_start(srow0[:], srow[D : D + 1, :])
                        recip0 = npool.tile([1, TQ], F32, tag=f"recip0{hh}")
                        nc.vector.reciprocal_approx_fast(recip0[0:1, :], srow0[0:1, :])
                        bcast = npool.tile([D, TQ], F32, tag=f"bcast{hh}")
                        nc.gpsimd.partition_broadcast(
                            bcast[:, :], recip0[0:1, :], channels=D
                        )
                        if hh == 0:
                            nc.vector.tensor_mul(
                                yT[0:D, pi, tqs], py[hh][0:D, :], bcast[:, :]
                            )
                        else:
                            y_tmp = npool.tile([D, TQ], mm_dt, tag="y_tmp")
                            nc.vector.tensor_mul(
                                y_tmp[:], py[hh][0:D, :], bcast[:, :]
                            )
                            nc.sync.dma_start(yT[D : 2 * D, pi, tqs], y_tmp[:])

        # ---------------- phase 3: output projection (partial) ----------------
        with (
            tc.tile_pool(name="ops2", bufs=3, space="PSUM") as ops2,
            tc.tile_pool(name="opool", bufs=3) as opool,
        ):
            for m in range(T // P):
                out_sb = opool.tile([P, C], F32, tag="osb")
                for n in range(2):
                    po = ops2.tile([P, 512], F32, tag="po")
                    for j in range(2):
                        nc.tensor.matmul(
                            po[:],
                            yT[:, j, m * P : (m + 1) * P],
                            wpT[:, j, n * 512 : (n + 1) * 512],
                            start=(j == 0),
                            stop=(j == 1),
                        )
                    nc.vector.tensor_copy(out_sb[:, n * 512 : (n + 1) * 512], po[:])
                nc.sync.dma_start(out[m * P : (m + 1) * P, :], out_sb[:])

    nc.finalize()
    return nc


_NC_CACHE = {}


def _get_nc(T=2048):
    if T not in _NC_CACHE:
        _NC_CACHE[T] = build(T=T)
    return _NC_CACHE[T]


def _make_in_maps(x, Wq, bq, Wk, bk, Wv, bv, Wp):
    in_maps = []
    for b in range(B):
        xb = np.ascontiguousarray(x[b], dtype=np.float32)
        for g in range(GROUPS):
            sl = slice(g * G, (g + 1) * G)
            in_maps.append(
                {
                    "x": xb,
                    "wq": np.ascontiguousarray(Wq[sl, :], dtype=np.float32),
                    "wk": np.ascontiguousarray(Wk[sl, :], dtype=np.float32),
                    "wv": np.ascontiguousarray(Wv[sl, :], dtype=np.float32),
                    "wp": np.ascontiguousarray(Wp[:, sl], dtype=np.float32),
                    "bq": np.ascontiguousarray(bq[sl], dtype=np.float32),
                    "bk": np.ascontiguousarray(bk[sl], dtype=np.float32),
                    "bv": np.ascontiguousarray(bv[sl], dtype=np.float32),
                }
            )
    return in_maps


def run(inputs, trace=False):
    """Run on 8 cores; returns (out [B,T,C] fp32, BassKernelResults)."""
    x = np.asarray(inputs["x"], dtype=np.float32)
    T = x.shape[1]
    in_maps = _make_in_maps(
        x,
        np.asarray(inputs["Wq"]), np.asarray(inputs["bq"]),
        np.asarray(inputs["Wk"]), np.asarray(inputs["bk"]),
        np.asarray(inputs["Wv"]), np.asarray(inputs["bv"]),
        np.asarray(inputs["Wp"]),
    )
    nc = _get_nc(T)
    res = run_bass_kernel_spmd(
        nc, in_maps, core_ids=list(range(B * GROUPS)), trace=trace
    )
    bp = np.asarray(inputs["bp"], dtype=np.float32)
    parts = [res.results[i]["out"] for i in range(B * GROUPS)]
    out = np.stack(
        [sum(parts[b * GROUPS : (b + 1) * GROUPS]) for b in range(B)]
    ) + bp[None, None, :]
    return out.astype(np.float32), res


def kernel(**inputs):
    out, _ = run(inputs, trace=False)
    return out
